# revision 24
# baseline (speedup 1.0000x reference)
"""CompressedLinear on 8 Trainium2 NeuronCores.

out[b,s,o] = sum_i x[b,s,i] * (w_int8[o,i] * scale[o]) + bias[o]
  x: [4, 2048, 4096] f32, w_int8: [16384, 4096] int32 (codes in [-64,63]),
  scale/bias: [16384] f32 -> out: [4, 2048, 16384] f32

Strategy (tensor-parallel over out_features):
  - Each of the 8 cores owns a 2048-row slice of W/scale/bias and computes
    out[:, :, c*2048:(c+1)*2048]; x is replicated.
  - Weight codes are exact in bf16; scale is applied AFTER the matmul
    (per-out-feature), so the matmul itself is integer-exact in bf16.
  - x is rounded to a single bf16 array: the only error is x's bf16
    rounding (~0.17% rel L2), far under the 2e-2 gate, at 1x bf16 matmul
    cost (an earlier hi/lo-split version paid 2x for precision nobody
    needed).
  - Per core loop: stationary operand = 128-token column block of x^T,
    moving operand = w^T; PSUM holds [128 tokens, 4x512 outfeat]; 32
    k-tiles x 4 banks = 128 matmuls per token tile, then a fused
    scale-mult + bias-add epilogue on DVE and a DMA store.
  - w is loaded in chunks so the first matmuls only wait on the chunk
    they read; the ramp-up is w-DMA-bound either way and the cold (HAM
    half-clock) matmuls hide inside that window.
  - The final token tile is split into two independent 2-bank PSUM tiles
    so the first half's epilogue+store overlaps the second half's
    matmuls, halving the serial tail after the last matmul.

All data layout transforms (transpose, int8->bf16 cast, scale/bias
broadcast) are host-side numpy; gather is a concat.
"""

import os

import numpy as np
import ml_dtypes

BF16 = ml_dtypes.bfloat16

OUT, IN = 16384, 4096
B, S = 4, 2048
TOK = B * S            # 8192 tokens
NCORES = 8
OSH = OUT // NCORES    # 2048 out-features per core
KT = IN // 128         # 32 k-tiles
TT = TOK // 128        # 64 token tiles
NB = OSH // 512        # 4 psum banks per token tile
NI8 = 12               # trailing k-tiles shipped as int8 + cast on-chip

_last_results = None   # BassKernelResults of the most recent run (for test.py)


def _build_program():
    from contextlib import ExitStack

    import concourse.bass as bass
    import concourse.tile as tile
    from concourse import mybir

    f32 = mybir.dt.float32
    bf16 = mybir.dt.bfloat16
    i8 = mybir.dt.int8

    nc = bass.Bass()
    xhi_d = nc.declare_dram_parameter("xhi", [TT, 128, KT, 128], bf16, isOutput=False)
    w_d = nc.declare_dram_parameter("w", [128, KT - NI8, OSH], bf16, isOutput=False)
    w8_d = nc.declare_dram_parameter("w8", [128, NI8, OSH], i8, isOutput=False)
    scale_d = nc.declare_dram_parameter("scale", [128, NB, 512], f32, isOutput=False)
    bias_d = nc.declare_dram_parameter("bias", [128, NB, 512], f32, isOutput=False)
    out_d = nc.declare_dram_parameter("out", [TT, 128, NB, 512], f32, isOutput=True)

    from concourse.tile import add_dep_helper

    with tile.TileContext(nc) as tc, ExitStack() as ctx:
        wpool = ctx.enter_context(tc.tile_pool(name="w", bufs=1))
        cpool = ctx.enter_context(tc.tile_pool(name="consts", bufs=1))
        xpool = ctx.enter_context(tc.tile_pool(name="x", bufs=2))
        opool = ctx.enter_context(tc.tile_pool(name="o", bufs=2))
        pspool = ctx.enter_context(tc.tile_pool(name="ps", bufs=2, space="PSUM"))

        # Hardware sync-wait slots are tiny (1 per PE LW/MM and per SWDGE
        # DMA, 2 per HWDGE DMA), and Tile's wait assignment is per-proc
        # minimal but not transitive. So every cross-engine dependency is
        # absorbed by a dedicated cheap "carrier" op on the consuming engine,
        # with explicit ordering edges so the scheduler keeps each carrier
        # ahead of its dependents and every instruction introduces at most
        # one new wait.
        def order(after, before):
            add_dep_helper(after.ins, before.ins, sync=False, reason="carrier order")

        # The ramp is w-load-DMA-bound, so the last NI8 k-tiles ship as
        # int8 (half the bytes) into a write-once staging tile and are cast
        # to bf16 on the otherwise-idle DVE/ACT engines. The int8 chunks
        # are interleaved into the bf16 HWDGE stream so no single w-wait
        # inside tile 0 exceeds the ~3.4us HAM re-throttle window.
        w_sb = wpool.tile([128, KT, OSH], bf16)
        wstage = wpool.tile([128, NI8, OSH], i8, tag="wstage")
        KB = KT - NI8  # bf16 k-tiles
        w_dmas = []
        i8_dmas = []
        for kind, k0, nk in (
            ("b", 0, 2), ("b", 2, 2), ("b", 4, 2), ("b", 6, 2),
            ("b", 8, 2), ("b", 10, 2), ("b", 12, 2), ("b", 14, 2),
            ("i", 0, 4), ("b", 16, 2), ("b", 18, 2), ("i", 4, 4),
            ("i", 8, 4),
        ):
            if kind == "b":
                w_dmas.append(
                    nc.sync.dma_start(
                        w_sb[:, k0 : k0 + nk, :], w_d[:, k0 : k0 + nk, :]
                    )
                )
            else:
                i8_dmas.append(
                    nc.sync.dma_start(
                        wstage[:, k0 : k0 + nk, :], w8_d[:, k0 : k0 + nk, :]
                    )
                )
        # 2-k-tile casts, alternating DVE/ACT; each waits only its chunk DMA.
        casts = []
        for c in range(NI8 // 2):
            src = wstage[:, 2 * c : 2 * c + 2, :]
            dst = w_sb[:, KB + 2 * c : KB + 2 * c + 2, :]
            if c % 2 == 0:
                casts.append(nc.vector.tensor_copy(dst, src))
            else:
                casts.append(nc.scalar.copy(dst, src))

        scale_sb = cpool.tile([128, NB, 512], f32, tag="scale")
        scale_dma = nc.sync.dma_start(scale_sb[:], scale_d[:])
        bias_sb = cpool.tile([128, NB, 512], f32, tag="bias")
        bias_dma = nc.sync.dma_start(bias_sb[:], bias_d[:])

        # Per-iteration disjoint scratch columns -> the carrier ops carry no
        # WAW deps of their own.
        scratch = cpool.tile([1, TT + 4], f32, tag="scratch")
        dummy = cpool.tile([1, 2 * TT + 4], f32, tag="dummy")
        dveA = cpool.tile([1, TT + 4], f32, tag="dveA")
        dveB = cpool.tile([1, TT + 4], f32, tag="dveB")
        # Preamble DVE carriers: observe the scale/bias const loads on DVE so
        # no steady-state DVE op pairs a DMAHW wait with another wait.
        pre = cpool.tile([1, 2], f32, tag="pre")
        nc.vector.tensor_copy(pre[:, 0:1], scale_sb[:1, 0, :1])
        nc.vector.tensor_copy(pre[:, 1:2], bias_sb[:1, 0, :1])

        psum_readers = []  # the last psum reader per sub-iteration
        last_mms = []  # final matmul per sub-iteration
        out_dmas = []
        out_copies = []
        x_dmas = []
        adds = []
        swdge = []  # every SWDGE dma in issue order (lane-sem coverage)
        hw_stores = []  # final stores routed via the HWDGE queue

        # Iteration plan: tiles 0..TT-2 process all NB banks at once; the
        # final tile is split into four 1-bank sub-iterations with separate
        # PSUM tiles, so each bank's epilogue overlaps the later banks'
        # matmuls. Each entry: (t, j0, nb, new_x, hw_store)
        plan = [(t, 0, NB, True, False) for t in range(TT - 1)]
        plan.append((TT - 1, 0, 1, True, False))
        plan.append((TT - 1, 1, 1, False, False))
        plan.append((TT - 1, 2, 1, False, False))
        plan.append((TT - 1, 3, 1, False, False))

        xhi = None
        for i, (t, j0, nb, new_x, hw_store) in enumerate(plan):
            if new_x:
                xhi = xpool.tile([128, KT, 128], bf16, tag="xhi")
                # POOL carrier chain, one wait each: gen-2 x-load DMA (its
                # lane sem would otherwise ride the new DMA as a WAW wait)
                # and gen-2 matmuls (x slot readers), before the x-slot
                # rewrite.
                ms1 = nc.gpsimd.memset(dummy[:, 2 * t : 2 * t + 1], 0)
                ms2 = nc.gpsimd.memset(dummy[:, 2 * t + 1 : 2 * t + 2], 0)
                order(ms2, ms1)
                if t >= 2:
                    add_dep_helper(
                        ms1.ins, x_dmas[t - 2].ins, reason="x WAW lane via carrier"
                    )
                    add_dep_helper(
                        ms2.ins,
                        last_mms[t - 2].ins,
                        reason="x slot reuse gated on POOL carrier",
                    )
                if t == 0:
                    # Split the first x tile into ascending k-ranges so the
                    # first matmul needs only a 0.13 MB piece, starting
                    # right after the SWDGE ucode boots. The SWDGE dynamic
                    # ring is 4 descriptors deep (DMA #n waits on #n-4), so
                    # the off-pattern waits these extra pieces cause in the
                    # next few SWDGE DMAs get explicit Pool carriers below.
                    x0_pieces = []
                    for ka, kb in ((0, 4), (4, 8), (8, 16), (16, KT)):
                        d1 = nc.gpsimd.dma_start(
                            xhi[:, ka:kb, :], xhi_d[t, :, ka:kb, :]
                        )
                        if x0_pieces:
                            order(d1, x0_pieces[-1])
                        x0_pieces.append(d1)
                        swdge.append(d1)
                else:
                    d1 = nc.gpsimd.dma_start(xhi[:], xhi_d[t])
                    swdge.append(d1)
                    if t == 1:
                        # d1(t1) is SWDGE #6 -> waits #2 (x0 piece 2).
                        add_dep_helper(
                            ms1.ins, x0_pieces[1].ins, reason="swdge ring-4"
                        )
                        order(d1, ms1)
                order(d1, ms2)
                x_dmas.append(d1)

            ps = pspool.tile([128, nb, 512], f32, tag="ps")
            # PE carrier: guard LDWEIGHTS absorbing the psum-slot-free (DVE)
            # wait so the first real matmul only waits on PE.
            guard = nc.tensor.ldweights(w_sb[:, 0, :128])
            if i >= 2:
                add_dep_helper(
                    guard.ins,
                    psum_readers[i - 2].ins,
                    reason="psum slot reuse gated on guard ldweights",
                )
            first_mm = None
            for k in range(KT):
                for j in range(nb):
                    mm = nc.tensor.matmul(
                        ps[:, j, :],
                        xhi[:, k, :],
                        w_sb[:, k, (j0 + j) * 512 : (j0 + j + 1) * 512],
                        start=(k == 0),
                        stop=(k == KT - 1),
                    )
                    if first_mm is None:
                        first_mm = mm
            order(first_mm, guard)
            last_mms.append(mm)

            ob = opool.tile([128, nb, 512], f32, tag="ob")
            # DVE carriers: absorb the ob-slot WAR deps (gen-2 out-store DMA
            # and gen-2 POOL scratch copy) ahead of the scale-mult.
            c1 = nc.vector.tensor_copy(dveA[:, i : i + 1], scale_sb[:1, 0, :1])
            c2 = nc.vector.tensor_copy(dveB[:, i : i + 1], scale_sb[:1, 0, :1])
            if i >= 2:
                add_dep_helper(
                    c1.ins, out_dmas[i - 2].ins, reason="ob reuse vs out dma"
                )
                add_dep_helper(
                    c2.ins, out_copies[i - 2].ins, reason="ob reuse vs pool copy"
                )
            mult = nc.vector.tensor_tensor(
                ob[:], ps[:], scale_sb[:, j0 : j0 + nb, :], mybir.AluOpType.mult
            )
            order(mult, c1)
            order(mult, c2)
            psum_readers.append(mult)
            adds.append(
                nc.vector.tensor_tensor(
                    ob[:], ob[:], bias_sb[:, j0 : j0 + nb, :], mybir.AluOpType.add
                )
            )
            if not hw_store:
                if t <= 1:
                    # od(t0) is SWDGE #5 -> waits #1; od(t1) is #7 -> #3:
                    # absorb the ring-4 wait on a Pool carrier.
                    msr = nc.gpsimd.memset(
                        dummy[:, 2 * TT + t : 2 * TT + t + 1], 0
                    )
                    add_dep_helper(
                        msr.ins, x0_pieces[2 * t].ins, reason="swdge ring-4"
                    )
                # POOL carrier: RAW on ob -> absorbs the DVE wait ahead of
                # the out-store.
                cp = nc.gpsimd.tensor_copy(scratch[:, i : i + 1], ob[:1, 0, :1])
                od = nc.gpsimd.dma_start(out_d[t, :, j0 : j0 + nb, :], ob[:])
                order(od, cp)
                if t <= 1:
                    order(od, msr)
                out_copies.append(cp)
                out_dmas.append(od)
                swdge.append(od)
            else:
                # Final stores ride the Activation engine's HWDGE queue:
                # its lane sems are untouched (no WAW wait), so the single
                # DVE wait is legal, and the Pool engine's expensive
                # dge-drain isn't gated on these stores.
                od = nc.scalar.dma_start(out_d[t, :, j0 : j0 + nb, :], ob[:])
                out_copies.append(adds[-1])
                out_dmas.append(od)
                hw_stores.append(od)

        # Tail carriers: SP nops, one wait each, observing every outstanding
        # sem (PE, DVE, Pool, all SWDGE lane sems, all HWDGE DMAs) so the
        # kernel-tail SP drain doesn't exceed its sync-wait slots.
        tail_deps = [
            last_mms[-1],
            adds[-1],
            out_copies[62],
            casts[-1],
            scale_dma,
            bias_dma,
            *w_dmas,
            *i8_dmas,
            *hw_stores,
        ]
        # SWDGE DMAs spread over 8 lane sems (assignment is not strictly
        # round-robin) -> observe a deep window of trailing DMAs.
        tail_deps += swdge[-20:]
        for i, dep in enumerate(tail_deps):
            nop = nc.engines[mybir.EngineType.SP].nop(
                nofuse=True, hint=f"tail_carrier_{i}"
            )
            add_dep_helper(nop.ins, dep.ins, reason="tail drain carrier")

    return nc


def kernel(x, weight_int8, scale, bias):
    global _last_results
    from concourse.bass_utils import run_bass_kernel_spmd

    x = np.asarray(x)
    weight_int8 = np.asarray(weight_int8)
    scale = np.asarray(scale, dtype=np.float32)
    bias = np.asarray(bias, dtype=np.float32)

    # x^T [IN, TOK] in bf16, tiled to [TT, 128p(IN), KT, 128(tok)]
    xT = np.ascontiguousarray(x.reshape(TOK, IN).astype(np.float32).T)
    x_hi = xT.astype(BF16)
    x_hi = np.ascontiguousarray(
        x_hi.reshape(KT, 128, TT, 128).transpose(2, 1, 0, 3)
    )

    in_maps = []
    for c in range(NCORES):
        wc = weight_int8[c * OSH : (c + 1) * OSH]
        # w^T [IN, OSH] tiled to [128p(IN), KT, OSH]
        wt = wc.T.reshape(KT, 128, OSH).transpose(1, 0, 2)
        wp = np.ascontiguousarray(wt[:, : KT - NI8, :].astype(np.float32)).astype(BF16)
        wp8 = np.ascontiguousarray(wt[:, KT - NI8 :, :]).astype(np.int8)
        sc = np.ascontiguousarray(
            np.broadcast_to(scale[c * OSH : (c + 1) * OSH], (128, OSH))
        ).reshape(128, NB, 512)
        bc = np.ascontiguousarray(
            np.broadcast_to(bias[c * OSH : (c + 1) * OSH], (128, OSH))
        ).reshape(128, NB, 512)
        in_maps.append({"xhi": x_hi, "w": wp, "w8": wp8, "scale": sc, "bias": bc})

    nc = _build_program()
    trace = bool(os.environ.get("KERNEL_TRACE"))
    kwargs = {}
    if trace:
        # Local-only profiling: stub the bucket upload and install the axon
        # NTFF hook (the image's antenv stub lacks axon_hooks).
        import sys
        import types

        from concourse import bass_utils as _bu

        _bu.upload_artifacts = lambda tmpdir: "local://" + tmpdir
        if "antenv.axon_hooks" not in sys.modules:
            import antenv

            mod = types.ModuleType("antenv.axon_hooks")
            _holder = [None]
            mod.set_axon_ntff_profile_hook = lambda h: _holder.__setitem__(0, h)
            mod.get_axon_ntff_profile_hook = lambda: _holder[0]
            antenv.axon_hooks = mod
            sys.modules["antenv.axon_hooks"] = mod
        from antenv.axon_hooks import (
            get_axon_ntff_profile_hook,
            set_axon_ntff_profile_hook,
        )

        if get_axon_ntff_profile_hook() is None:
            from trn_agent_boot.trn_boot import _ntff_profile_via_ctypes

            set_axon_ntff_profile_hook(
                _ntff_profile_via_ctypes(
                    os.environ.get("PJRT_LIBRARY_PATH", "/opt/axon/libaxon_pjrt.so")
                )
            )
        tmpdir = os.environ.get("KERNEL_TRACE_DIR")
        if tmpdir:
            os.makedirs(tmpdir, exist_ok=True)
            kwargs["tmpdir"] = tmpdir

    res = run_bass_kernel_spmd(
        nc,
        in_maps,
        list(range(NCORES)),
        trace=trace,
        **kwargs,
    )
    _last_results = res

    parts = [res.results[c]["out"].reshape(TOK, OSH) for c in range(NCORES)]
    return np.concatenate(parts, axis=1).reshape(B, S, OUT)


# revision 25
# speedup vs baseline: 1.2021x; 1.2021x over previous
"""CompressedLinear on 8 Trainium2 NeuronCores.

out[b,s,o] = sum_i x[b,s,i] * (w_int8[o,i] * scale[o]) + bias[o]
  x: [4, 2048, 4096] f32, w_int8: [16384, 4096] int32 (codes in [-64,63]),
  scale/bias: [16384] f32 -> out: [4, 2048, 16384] f32

Strategy (tensor-parallel over out_features):
  - Each of the 8 cores owns a 2048-row slice of W/scale/bias and computes
    out[:, :, c*2048:(c+1)*2048]; x is replicated.
  - Weight codes are exact in bf16; scale is applied AFTER the matmul
    (per-out-feature), so the matmul itself is integer-exact in bf16.
  - x is rounded to a single bf16 array: the only error is x's bf16
    rounding (~0.17% rel L2), far under the 2e-2 gate, at 1x bf16 matmul
    cost (an earlier hi/lo-split version paid 2x for precision nobody
    needed).
  - Per core loop: stationary operand = 128-token column block of x^T,
    moving operand = w^T; PSUM holds [128 tokens, 4x512 outfeat]; 32
    k-tiles x 4 banks = 128 matmuls per token tile, then a fused
    scale-mult + bias-add epilogue on DVE and a DMA store.
  - w is loaded in chunks so the first matmuls only wait on the chunk
    they read; the ramp-up is w-DMA-bound either way and the cold (HAM
    half-clock) matmuls hide inside that window.
  - The final token tile is split into two independent 2-bank PSUM tiles
    so the first half's epilogue+store overlaps the second half's
    matmuls, halving the serial tail after the last matmul.

All data layout transforms (transpose, int8->bf16 cast, scale/bias
broadcast) are host-side numpy; gather is a concat.
"""

import os

import numpy as np
import ml_dtypes

BF16 = ml_dtypes.bfloat16

OUT, IN = 16384, 4096
B, S = 4, 2048
TOK = B * S            # 8192 tokens
NCORES = 8
OSH = OUT // NCORES    # 2048 out-features per core
KT = IN // 128         # 32 k-tiles
TT = TOK // 128        # 64 token tiles
NB = OSH // 512        # 4 psum banks per token tile
NI8 = 12               # trailing k-tiles shipped as int8 + cast on-chip

_last_results = None   # BassKernelResults of the most recent run (for test.py)


def _build_program():
    from contextlib import ExitStack

    import concourse.bass as bass
    import concourse.tile as tile
    from concourse import mybir

    f32 = mybir.dt.float32
    bf16 = mybir.dt.bfloat16
    i8 = mybir.dt.int8

    nc = bass.Bass()
    xhi_d = nc.declare_dram_parameter("xhi", [TT, 128, KT, 128], bf16, isOutput=False)
    w_d = nc.declare_dram_parameter("w", [128, KT - NI8, OSH], bf16, isOutput=False)
    w8_d = nc.declare_dram_parameter("w8", [128, NI8, OSH], i8, isOutput=False)
    scale_d = nc.declare_dram_parameter("scale", [128, NB, 512], f32, isOutput=False)
    bias_d = nc.declare_dram_parameter("bias", [128, NB, 512], f32, isOutput=False)
    out_d = nc.declare_dram_parameter("out", [TT, 128, NB, 512], f32, isOutput=True)

    from concourse.tile import add_dep_helper

    with tile.TileContext(nc) as tc, ExitStack() as ctx:
        wpool = ctx.enter_context(tc.tile_pool(name="w", bufs=1))
        cpool = ctx.enter_context(tc.tile_pool(name="consts", bufs=1))
        xpool = ctx.enter_context(tc.tile_pool(name="x", bufs=2))
        opool = ctx.enter_context(tc.tile_pool(name="o", bufs=2))
        pspool = ctx.enter_context(tc.tile_pool(name="ps", bufs=2, space="PSUM"))

        # Hardware sync-wait slots are tiny (1 per PE LW/MM and per SWDGE
        # DMA, 2 per HWDGE DMA), and Tile's wait assignment is per-proc
        # minimal but not transitive. So every cross-engine dependency is
        # absorbed by a dedicated cheap "carrier" op on the consuming engine,
        # with explicit ordering edges so the scheduler keeps each carrier
        # ahead of its dependents and every instruction introduces at most
        # one new wait.
        def order(after, before):
            add_dep_helper(after.ins, before.ins, sync=False, reason="carrier order")

        # The ramp is w-load-DMA-bound, so the last NI8 k-tiles ship as
        # int8 (half the bytes) into a write-once staging tile and are cast
        # to bf16 on the otherwise-idle DVE/ACT engines. The int8 chunks
        # are interleaved into the bf16 HWDGE stream so no single w-wait
        # inside tile 0 exceeds the ~3.4us HAM re-throttle window.
        # x tile 0 loads first on the SP HWDGE queue (1 MB, ~3us) so the
        # matmul stream starts as early as possible; the w chunks follow.
        x0_sb = xpool.tile([128, KT, 128], bf16, tag="xhi")
        x0_dma = nc.sync.dma_start(x0_sb[:], xhi_d[0])

        w_sb = wpool.tile([128, KT, OSH], bf16)
        wstage = wpool.tile([128, NI8, OSH], i8, tag="wstage")
        KB = KT - NI8  # bf16 k-tiles
        w_dmas = []
        i8_dmas = []
        for kind, k0, nk in (
            ("b", 0, 2), ("b", 2, 2), ("b", 4, 2), ("b", 6, 2),
            ("b", 8, 2), ("b", 10, 2), ("b", 12, 2), ("b", 14, 2),
            ("i", 0, 4), ("b", 16, 2), ("b", 18, 2), ("i", 4, 4),
            ("i", 8, 4),
        ):
            if kind == "b":
                w_dmas.append(
                    nc.sync.dma_start(
                        w_sb[:, k0 : k0 + nk, :], w_d[:, k0 : k0 + nk, :]
                    )
                )
            else:
                i8_dmas.append(
                    nc.sync.dma_start(
                        wstage[:, k0 : k0 + nk, :], w8_d[:, k0 : k0 + nk, :]
                    )
                )
        # 2-k-tile casts, alternating DVE/ACT; each waits only its chunk DMA.
        casts = []
        for c in range(NI8 // 2):
            src = wstage[:, 2 * c : 2 * c + 2, :]
            dst = w_sb[:, KB + 2 * c : KB + 2 * c + 2, :]
            if c % 2 == 0:
                casts.append(nc.vector.tensor_copy(dst, src))
            else:
                casts.append(nc.scalar.copy(dst, src))

        scale_sb = cpool.tile([128, NB, 512], f32, tag="scale")
        scale_dma = nc.sync.dma_start(scale_sb[:], scale_d[:])
        bias_sb = cpool.tile([128, NB, 512], f32, tag="bias")
        bias_dma = nc.sync.dma_start(bias_sb[:], bias_d[:])

        # Per-iteration disjoint scratch columns -> the carrier ops carry no
        # WAW deps of their own.
        scratch = cpool.tile([1, TT + 4], f32, tag="scratch")
        dummy = cpool.tile([1, 2 * TT + 4], f32, tag="dummy")
        dveA = cpool.tile([1, TT + 4], f32, tag="dveA")
        dveB = cpool.tile([1, TT + 4], f32, tag="dveB")
        # Preamble DVE carriers: observe the scale/bias const loads on DVE so
        # no steady-state DVE op pairs a DMAHW wait with another wait.
        pre = cpool.tile([1, 2], f32, tag="pre")
        nc.vector.tensor_copy(pre[:, 0:1], scale_sb[:1, 0, :1])
        nc.vector.tensor_copy(pre[:, 1:2], bias_sb[:1, 0, :1])

        psum_readers = []  # the last psum reader per sub-iteration
        last_mms = []  # final matmul per sub-iteration
        out_dmas = []
        out_copies = []
        x_dmas = []
        adds = []
        swdge = []  # every SWDGE dma in issue order (lane-sem coverage)
        hw_stores = []  # final stores routed via the HWDGE queue

        # Iteration plan: tiles 0..TT-2 process all NB banks at once; the
        # final tile is split into four 1-bank sub-iterations with separate
        # PSUM tiles, so each bank's epilogue overlaps the later banks'
        # matmuls. Each entry: (t, j0, nb, new_x, hw_store)
        plan = [(t, 0, NB, True, False) for t in range(TT - 1)]
        plan.append((TT - 1, 0, 1, True, False))
        plan.append((TT - 1, 1, 1, False, False))
        plan.append((TT - 1, 2, 1, False, False))
        plan.append((TT - 1, 3, 1, False, False))

        xhi = None
        for i, (t, j0, nb, new_x, hw_store) in enumerate(plan):
            if new_x and t == 0:
                xhi = x0_sb
                x_dmas.append(x0_dma)
            elif new_x:
                xhi = xpool.tile([128, KT, 128], bf16, tag="xhi")
                # POOL carrier chain, one wait each: gen-2 x-load DMA (its
                # lane sem would otherwise ride the new DMA as a WAW wait)
                # and gen-2 matmuls (x slot readers), before the x-slot
                # rewrite.
                ms1 = nc.gpsimd.memset(dummy[:, 2 * t : 2 * t + 1], 0)
                ms2 = nc.gpsimd.memset(dummy[:, 2 * t + 1 : 2 * t + 2], 0)
                order(ms2, ms1)
                if t >= 2:
                    add_dep_helper(
                        ms1.ins, x_dmas[t - 2].ins, reason="x WAW lane via carrier"
                    )
                    add_dep_helper(
                        ms2.ins,
                        last_mms[t - 2].ins,
                        reason="x slot reuse gated on POOL carrier",
                    )
                d1 = nc.gpsimd.dma_start(xhi[:], xhi_d[t])
                swdge.append(d1)
                order(d1, ms2)
                x_dmas.append(d1)

            ps = pspool.tile([128, nb, 512], f32, tag="ps")
            # PE carrier: guard LDWEIGHTS absorbing the psum-slot-free (DVE)
            # wait so the first real matmul only waits on PE.
            guard = nc.tensor.ldweights(w_sb[:, 0, :128])
            if i >= 2:
                add_dep_helper(
                    guard.ins,
                    psum_readers[i - 2].ins,
                    reason="psum slot reuse gated on guard ldweights",
                )
            first_mm = None
            for k in range(KT):
                for j in range(nb):
                    mm = nc.tensor.matmul(
                        ps[:, j, :],
                        xhi[:, k, :],
                        w_sb[:, k, (j0 + j) * 512 : (j0 + j + 1) * 512],
                        start=(k == 0),
                        stop=(k == KT - 1),
                    )
                    if first_mm is None:
                        first_mm = mm
            order(first_mm, guard)
            last_mms.append(mm)

            ob = opool.tile([128, nb, 512], f32, tag="ob")
            # DVE carriers: absorb the ob-slot WAR deps (gen-2 out-store DMA
            # and gen-2 POOL scratch copy) ahead of the scale-mult.
            c1 = nc.vector.tensor_copy(dveA[:, i : i + 1], scale_sb[:1, 0, :1])
            c2 = nc.vector.tensor_copy(dveB[:, i : i + 1], scale_sb[:1, 0, :1])
            if i >= 2:
                add_dep_helper(
                    c1.ins, out_dmas[i - 2].ins, reason="ob reuse vs out dma"
                )
                add_dep_helper(
                    c2.ins, out_copies[i - 2].ins, reason="ob reuse vs pool copy"
                )
            mult = nc.vector.tensor_tensor(
                ob[:], ps[:], scale_sb[:, j0 : j0 + nb, :], mybir.AluOpType.mult
            )
            order(mult, c1)
            order(mult, c2)
            psum_readers.append(mult)
            adds.append(
                nc.vector.tensor_tensor(
                    ob[:], ob[:], bias_sb[:, j0 : j0 + nb, :], mybir.AluOpType.add
                )
            )
            if not hw_store:
                # POOL carrier: RAW on ob -> absorbs the DVE wait ahead of
                # the out-store.
                cp = nc.gpsimd.tensor_copy(scratch[:, i : i + 1], ob[:1, 0, :1])
                od = nc.gpsimd.dma_start(out_d[t, :, j0 : j0 + nb, :], ob[:])
                order(od, cp)
                out_copies.append(cp)
                out_dmas.append(od)
                swdge.append(od)
            else:
                # Final stores ride the Activation engine's HWDGE queue:
                # its lane sems are untouched (no WAW wait), so the single
                # DVE wait is legal, and the Pool engine's expensive
                # dge-drain isn't gated on these stores.
                od = nc.scalar.dma_start(out_d[t, :, j0 : j0 + nb, :], ob[:])
                out_copies.append(adds[-1])
                out_dmas.append(od)
                hw_stores.append(od)

        # Tail carriers: SP nops, one wait each, observing every outstanding
        # sem (PE, DVE, Pool, all SWDGE lane sems, all HWDGE DMAs) so the
        # kernel-tail SP drain doesn't exceed its sync-wait slots.
        tail_deps = [
            last_mms[-1],
            adds[-1],
            out_copies[62],
            casts[-1],
            scale_dma,
            bias_dma,
            *w_dmas,
            *i8_dmas,
            *hw_stores,
        ]
        # SWDGE DMAs spread over 8 lane sems (assignment is not strictly
        # round-robin) -> observe a deep window of trailing DMAs.
        tail_deps += swdge[-20:]
        for i, dep in enumerate(tail_deps):
            nop = nc.engines[mybir.EngineType.SP].nop(
                nofuse=True, hint=f"tail_carrier_{i}"
            )
            add_dep_helper(nop.ins, dep.ins, reason="tail drain carrier")

    return nc


def kernel(x, weight_int8, scale, bias):
    global _last_results
    from concourse.bass_utils import run_bass_kernel_spmd

    x = np.asarray(x)
    weight_int8 = np.asarray(weight_int8)
    scale = np.asarray(scale, dtype=np.float32)
    bias = np.asarray(bias, dtype=np.float32)

    # x^T [IN, TOK] in bf16, tiled to [TT, 128p(IN), KT, 128(tok)]
    xT = np.ascontiguousarray(x.reshape(TOK, IN).astype(np.float32).T)
    x_hi = xT.astype(BF16)
    x_hi = np.ascontiguousarray(
        x_hi.reshape(KT, 128, TT, 128).transpose(2, 1, 0, 3)
    )

    in_maps = []
    for c in range(NCORES):
        wc = weight_int8[c * OSH : (c + 1) * OSH]
        # w^T [IN, OSH] tiled to [128p(IN), KT, OSH]
        wt = wc.T.reshape(KT, 128, OSH).transpose(1, 0, 2)
        wp = np.ascontiguousarray(wt[:, : KT - NI8, :].astype(np.float32)).astype(BF16)
        wp8 = np.ascontiguousarray(wt[:, KT - NI8 :, :]).astype(np.int8)
        sc = np.ascontiguousarray(
            np.broadcast_to(scale[c * OSH : (c + 1) * OSH], (128, OSH))
        ).reshape(128, NB, 512)
        bc = np.ascontiguousarray(
            np.broadcast_to(bias[c * OSH : (c + 1) * OSH], (128, OSH))
        ).reshape(128, NB, 512)
        in_maps.append({"xhi": x_hi, "w": wp, "w8": wp8, "scale": sc, "bias": bc})

    nc = _build_program()
    trace = bool(os.environ.get("KERNEL_TRACE"))
    kwargs = {}
    if trace:
        # Local-only profiling: stub the bucket upload and install the axon
        # NTFF hook (the image's antenv stub lacks axon_hooks).
        import sys
        import types

        from concourse import bass_utils as _bu

        _bu.upload_artifacts = lambda tmpdir: "local://" + tmpdir
        if "antenv.axon_hooks" not in sys.modules:
            import antenv

            mod = types.ModuleType("antenv.axon_hooks")
            _holder = [None]
            mod.set_axon_ntff_profile_hook = lambda h: _holder.__setitem__(0, h)
            mod.get_axon_ntff_profile_hook = lambda: _holder[0]
            antenv.axon_hooks = mod
            sys.modules["antenv.axon_hooks"] = mod
        from antenv.axon_hooks import (
            get_axon_ntff_profile_hook,
            set_axon_ntff_profile_hook,
        )

        if get_axon_ntff_profile_hook() is None:
            from trn_agent_boot.trn_boot import _ntff_profile_via_ctypes

            set_axon_ntff_profile_hook(
                _ntff_profile_via_ctypes(
                    os.environ.get("PJRT_LIBRARY_PATH", "/opt/axon/libaxon_pjrt.so")
                )
            )
        tmpdir = os.environ.get("KERNEL_TRACE_DIR")
        if tmpdir:
            os.makedirs(tmpdir, exist_ok=True)
            kwargs["tmpdir"] = tmpdir

    res = run_bass_kernel_spmd(
        nc,
        in_maps,
        list(range(NCORES)),
        trace=trace,
        **kwargs,
    )
    _last_results = res

    parts = [res.results[c]["out"].reshape(TOK, OSH) for c in range(NCORES)]
    return np.concatenate(parts, axis=1).reshape(B, S, OUT)


# revision 27
# speedup vs baseline: 1.2027x; 1.0005x over previous
"""CompressedLinear on 8 Trainium2 NeuronCores.

out[b,s,o] = sum_i x[b,s,i] * (w_int8[o,i] * scale[o]) + bias[o]
  x: [4, 2048, 4096] f32, w_int8: [16384, 4096] int32 (codes in [-64,63]),
  scale/bias: [16384] f32 -> out: [4, 2048, 16384] f32

Strategy (tensor-parallel over out_features):
  - Each of the 8 cores owns a 2048-row slice of W/scale/bias and computes
    out[:, :, c*2048:(c+1)*2048]; x is replicated.
  - Weight codes are exact in bf16; scale is applied AFTER the matmul
    (per-out-feature), so the matmul itself is integer-exact in bf16.
  - x is rounded to a single bf16 array: the only error is x's bf16
    rounding (~0.17% rel L2), far under the 2e-2 gate, at 1x bf16 matmul
    cost (an earlier hi/lo-split version paid 2x for precision nobody
    needed).
  - Per core loop: stationary operand = 128-token column block of x^T,
    moving operand = w^T; PSUM holds [128 tokens, 4x512 outfeat]; 32
    k-tiles x 4 banks = 128 matmuls per token tile, then a fused
    scale-mult + bias-add epilogue on DVE and a DMA store.
  - w is loaded in chunks so the first matmuls only wait on the chunk
    they read; the ramp-up is w-DMA-bound either way and the cold (HAM
    half-clock) matmuls hide inside that window.
  - The final token tile is split into two independent 2-bank PSUM tiles
    so the first half's epilogue+store overlaps the second half's
    matmuls, halving the serial tail after the last matmul.

All data layout transforms (transpose, int8->bf16 cast, scale/bias
broadcast) are host-side numpy; gather is a concat.
"""

import os

import numpy as np
import ml_dtypes

BF16 = ml_dtypes.bfloat16

OUT, IN = 16384, 4096
B, S = 4, 2048
TOK = B * S            # 8192 tokens
NCORES = 8
OSH = OUT // NCORES    # 2048 out-features per core
KT = IN // 128         # 32 k-tiles
TT = TOK // 128        # 64 token tiles
NB = OSH // 512        # 4 psum banks per token tile
NI8 = 12               # trailing k-tiles shipped as int8 + cast on-chip

_last_results = None   # BassKernelResults of the most recent run (for test.py)


def _build_program():
    from contextlib import ExitStack

    import concourse.bass as bass
    import concourse.tile as tile
    from concourse import mybir

    f32 = mybir.dt.float32
    bf16 = mybir.dt.bfloat16
    i8 = mybir.dt.int8

    nc = bass.Bass()
    xhi_d = nc.declare_dram_parameter("xhi", [TT, 128, KT, 128], bf16, isOutput=False)
    w_d = nc.declare_dram_parameter("w", [128, KT - NI8, OSH], bf16, isOutput=False)
    w8_d = nc.declare_dram_parameter("w8", [128, NI8, OSH], i8, isOutput=False)
    scale_d = nc.declare_dram_parameter("scale", [128, NB, 512], f32, isOutput=False)
    bias_d = nc.declare_dram_parameter("bias", [128, NB, 512], f32, isOutput=False)
    out_d = nc.declare_dram_parameter("out", [TT, 128, NB, 512], f32, isOutput=True)

    from concourse.tile import add_dep_helper

    with tile.TileContext(nc) as tc, ExitStack() as ctx:
        wpool = ctx.enter_context(tc.tile_pool(name="w", bufs=1))
        cpool = ctx.enter_context(tc.tile_pool(name="consts", bufs=1))
        xpool = ctx.enter_context(tc.tile_pool(name="x", bufs=2))
        opool = ctx.enter_context(tc.tile_pool(name="o", bufs=2))
        pspool = ctx.enter_context(tc.tile_pool(name="ps", bufs=2, space="PSUM"))

        # Hardware sync-wait slots are tiny (1 per PE LW/MM and per SWDGE
        # DMA, 2 per HWDGE DMA), and Tile's wait assignment is per-proc
        # minimal but not transitive. So every cross-engine dependency is
        # absorbed by a dedicated cheap "carrier" op on the consuming engine,
        # with explicit ordering edges so the scheduler keeps each carrier
        # ahead of its dependents and every instruction introduces at most
        # one new wait.
        def order(after, before):
            add_dep_helper(after.ins, before.ins, sync=False, reason="carrier order")

        # The ramp is w-load-DMA-bound, so the last NI8 k-tiles ship as
        # int8 (half the bytes) into a write-once staging tile and are cast
        # to bf16 on the otherwise-idle DVE/ACT engines. The int8 chunks
        # are interleaved into the bf16 HWDGE stream so no single w-wait
        # inside tile 0 exceeds the ~3.4us HAM re-throttle window.
        # x tile 0 loads first on the SP HWDGE queue (1 MB, ~3us) so the
        # matmul stream starts as early as possible; the w chunks follow.
        x0_sb = xpool.tile([128, KT, 128], bf16, tag="xhi")
        x0_dma = nc.sync.dma_start(x0_sb[:], xhi_d[0])

        w_sb = wpool.tile([128, KT, OSH], bf16)
        wstage = wpool.tile([128, NI8, OSH], i8, tag="wstage")
        KB = KT - NI8  # bf16 k-tiles
        w_dmas = []
        i8_dmas = []
        for kind, k0, nk in (
            ("b", 0, 2), ("b", 2, 2), ("b", 4, 2), ("b", 6, 2),
            ("b", 8, 2), ("b", 10, 2), ("b", 12, 2), ("b", 14, 2),
            ("i", 0, 4), ("b", 16, 2), ("b", 18, 2), ("i", 4, 4),
            ("i", 8, 4),
        ):
            if kind == "b":
                w_dmas.append(
                    nc.sync.dma_start(
                        w_sb[:, k0 : k0 + nk, :], w_d[:, k0 : k0 + nk, :]
                    )
                )
            else:
                i8_dmas.append(
                    nc.sync.dma_start(
                        wstage[:, k0 : k0 + nk, :], w8_d[:, k0 : k0 + nk, :]
                    )
                )
        # 2-k-tile casts, alternating DVE/ACT; each waits only its chunk DMA.
        casts = []
        for c in range(NI8 // 2):
            src = wstage[:, 2 * c : 2 * c + 2, :]
            dst = w_sb[:, KB + 2 * c : KB + 2 * c + 2, :]
            if c % 2 == 0:
                casts.append(nc.vector.tensor_copy(dst, src))
            else:
                casts.append(nc.scalar.copy(dst, src))

        scale_sb = cpool.tile([128, NB, 512], f32, tag="scale")
        scale_dma = nc.sync.dma_start(scale_sb[:], scale_d[:])
        bias_sb = cpool.tile([128, NB, 512], f32, tag="bias")
        bias_dma = nc.sync.dma_start(bias_sb[:], bias_d[:])

        # Per-iteration disjoint scratch columns -> the carrier ops carry no
        # WAW deps of their own.
        scratch = cpool.tile([1, TT + 4], f32, tag="scratch")
        dummy = cpool.tile([1, 2 * TT + 4], f32, tag="dummy")
        dveA = cpool.tile([1, TT + 4], f32, tag="dveA")
        dveB = cpool.tile([1, TT + 4], f32, tag="dveB")
        # Preamble DVE carriers: observe the scale/bias const loads on DVE so
        # no steady-state DVE op pairs a DMAHW wait with another wait.
        pre = cpool.tile([1, 2], f32, tag="pre")
        nc.vector.tensor_copy(pre[:, 0:1], scale_sb[:1, 0, :1])
        nc.vector.tensor_copy(pre[:, 1:2], bias_sb[:1, 0, :1])

        psum_readers = []  # the last psum reader per sub-iteration
        last_mms = []  # final matmul per sub-iteration
        out_dmas = []
        out_copies = []
        x_dmas = []
        adds = []
        swdge = []  # every SWDGE dma in issue order (lane-sem coverage)
        hw_stores = []  # final stores routed via the HWDGE queue

        # Iteration plan: tiles 0..TT-2 process all NB banks at once; the
        # final tile is split into four 1-bank sub-iterations with separate
        # PSUM tiles, so each bank's epilogue overlaps the later banks'
        # matmuls. Each entry: (t, j0, nb, new_x, hw_store)
        plan = [(t, 0, NB, True, False) for t in range(TT - 1)]
        plan.append((TT - 1, 0, 1, True, False))
        plan.append((TT - 1, 1, 1, False, False))
        plan.append((TT - 1, 2, 1, False, False))
        plan.append((TT - 1, 3, 1, False, False))

        xhi = None
        for i, (t, j0, nb, new_x, hw_store) in enumerate(plan):
            if new_x and t == 0:
                xhi = x0_sb
                x_dmas.append(x0_dma)
            elif new_x:
                xhi = xpool.tile([128, KT, 128], bf16, tag="xhi")
                # POOL carrier chain, one wait each: gen-2 x-load DMA (its
                # lane sem would otherwise ride the new DMA as a WAW wait)
                # and gen-2 matmuls (x slot readers), before the x-slot
                # rewrite.
                ms1 = nc.gpsimd.memset(dummy[:, 2 * t : 2 * t + 1], 0)
                ms2 = nc.gpsimd.memset(dummy[:, 2 * t + 1 : 2 * t + 2], 0)
                order(ms2, ms1)
                if t >= 2:
                    add_dep_helper(
                        ms1.ins, x_dmas[t - 2].ins, reason="x WAW lane via carrier"
                    )
                    add_dep_helper(
                        ms2.ins,
                        last_mms[t - 2].ins,
                        reason="x slot reuse gated on POOL carrier",
                    )
                d1 = nc.gpsimd.dma_start(xhi[:], xhi_d[t])
                swdge.append(d1)
                order(d1, ms2)
                x_dmas.append(d1)

            ps = pspool.tile([128, nb, 512], f32, tag="ps")
            # PE carrier: guard LDWEIGHTS absorbing the psum-slot-free (DVE)
            # wait so the first real matmul only waits on PE.
            guard = nc.tensor.ldweights(w_sb[:, 0, :128])
            if i >= 2:
                add_dep_helper(
                    guard.ins,
                    psum_readers[i - 2].ins,
                    reason="psum slot reuse gated on guard ldweights",
                )
            first_mm = None
            for k in range(KT):
                for j in range(nb):
                    mm = nc.tensor.matmul(
                        ps[:, j, :],
                        xhi[:, k, :],
                        w_sb[:, k, (j0 + j) * 512 : (j0 + j + 1) * 512],
                        start=(k == 0),
                        stop=(k == KT - 1),
                    )
                    if first_mm is None:
                        first_mm = mm
            order(first_mm, guard)
            last_mms.append(mm)

            ob = opool.tile([128, nb, 512], f32, tag="ob")
            # DVE carriers: absorb the ob-slot WAR deps (gen-2 out-store DMA
            # and gen-2 POOL scratch copy) ahead of the scale-mult.
            c1 = nc.vector.tensor_copy(dveA[:, i : i + 1], scale_sb[:1, 0, :1])
            c2 = nc.vector.tensor_copy(dveB[:, i : i + 1], scale_sb[:1, 0, :1])
            if i >= 2:
                add_dep_helper(
                    c1.ins, out_dmas[i - 2].ins, reason="ob reuse vs out dma"
                )
                add_dep_helper(
                    c2.ins, out_copies[i - 2].ins, reason="ob reuse vs pool copy"
                )
            mult = nc.vector.tensor_tensor(
                ob[:], ps[:], scale_sb[:, j0 : j0 + nb, :], mybir.AluOpType.mult
            )
            order(mult, c1)
            order(mult, c2)
            psum_readers.append(mult)
            adds.append(
                nc.vector.tensor_tensor(
                    ob[:], ob[:], bias_sb[:, j0 : j0 + nb, :], mybir.AluOpType.add
                )
            )
            if not hw_store:
                # POOL carrier: RAW on ob -> absorbs the DVE wait ahead of
                # the out-store.
                cp = nc.gpsimd.tensor_copy(scratch[:, i : i + 1], ob[:1, 0, :1])
                od = nc.gpsimd.dma_start(out_d[t, :, j0 : j0 + nb, :], ob[:])
                order(od, cp)
                out_copies.append(cp)
                out_dmas.append(od)
                swdge.append(od)
            else:
                # Final stores ride the Activation engine's HWDGE queue:
                # its lane sems are untouched (no WAW wait), so the single
                # DVE wait is legal, and the Pool engine's expensive
                # dge-drain isn't gated on these stores.
                od = nc.scalar.dma_start(out_d[t, :, j0 : j0 + nb, :], ob[:])
                out_copies.append(adds[-1])
                out_dmas.append(od)
                hw_stores.append(od)

        # Tail carriers: SP nops, one wait each, observing every outstanding
        # sem (PE, DVE, Pool, all SWDGE lane sems, all HWDGE DMAs) so the
        # kernel-tail SP drain doesn't exceed its sync-wait slots.
        tail_deps = [
            last_mms[-1],
            adds[-1],
            out_copies[62],
            casts[-1],
            scale_dma,
            bias_dma,
            *w_dmas,
            *i8_dmas,
            *hw_stores,
        ]
        # SWDGE DMAs spread over 8 lane sems (assignment is not strictly
        # round-robin) -> observe a deep window of trailing DMAs.
        tail_deps += swdge[-20:]
        for i, dep in enumerate(tail_deps):
            nop = nc.engines[mybir.EngineType.SP].nop(
                nofuse=True, hint=f"tail_carrier_{i}"
            )
            add_dep_helper(nop.ins, dep.ins, reason="tail drain carrier")

    return nc


def kernel(x, weight_int8, scale, bias):
    global _last_results
    from concourse.bass_utils import run_bass_kernel_spmd

    x = np.asarray(x)
    weight_int8 = np.asarray(weight_int8)
    scale = np.asarray(scale, dtype=np.float32)
    bias = np.asarray(bias, dtype=np.float32)

    # x^T [IN, TOK] in bf16, tiled to [TT, 128p(IN), KT, 128(tok)]
    xT = np.ascontiguousarray(x.reshape(TOK, IN).astype(np.float32).T)
    x_hi = xT.astype(BF16)
    x_hi = np.ascontiguousarray(
        x_hi.reshape(KT, 128, TT, 128).transpose(2, 1, 0, 3)
    )

    in_maps = []
    for c in range(NCORES):
        wc = weight_int8[c * OSH : (c + 1) * OSH]
        # w^T [IN, OSH] tiled to [128p(IN), KT, OSH]
        wt = wc.T.reshape(KT, 128, OSH).transpose(1, 0, 2)
        wp = np.ascontiguousarray(wt[:, : KT - NI8, :].astype(np.float32)).astype(BF16)
        wp8 = np.ascontiguousarray(wt[:, KT - NI8 :, :]).astype(np.int8)
        sc = np.ascontiguousarray(
            np.broadcast_to(scale[c * OSH : (c + 1) * OSH], (128, OSH))
        ).reshape(128, NB, 512)
        bc = np.ascontiguousarray(
            np.broadcast_to(bias[c * OSH : (c + 1) * OSH], (128, OSH))
        ).reshape(128, NB, 512)
        in_maps.append({"xhi": x_hi, "w": wp, "w8": wp8, "scale": sc, "bias": bc})

    nc = _build_program()
    trace = bool(os.environ.get("KERNEL_TRACE"))
    kwargs = {}
    if trace:
        # Local-only profiling: stub the bucket upload and install the axon
        # NTFF hook (the image's antenv stub lacks axon_hooks).
        import sys
        import types

        from concourse import bass_utils as _bu

        _bu.upload_artifacts = lambda tmpdir: "local://" + tmpdir
        if "antenv.axon_hooks" not in sys.modules:
            import antenv

            mod = types.ModuleType("antenv.axon_hooks")
            _holder = [None]
            mod.set_axon_ntff_profile_hook = lambda h: _holder.__setitem__(0, h)
            mod.get_axon_ntff_profile_hook = lambda: _holder[0]
            antenv.axon_hooks = mod
            sys.modules["antenv.axon_hooks"] = mod
        from antenv.axon_hooks import (
            get_axon_ntff_profile_hook,
            set_axon_ntff_profile_hook,
        )

        if get_axon_ntff_profile_hook() is None:
            from trn_agent_boot.trn_boot import _ntff_profile_via_ctypes

            set_axon_ntff_profile_hook(
                _ntff_profile_via_ctypes(
                    os.environ.get("PJRT_LIBRARY_PATH", "/opt/axon/libaxon_pjrt.so")
                )
            )
        tmpdir = os.environ.get("KERNEL_TRACE_DIR")
        if tmpdir:
            os.makedirs(tmpdir, exist_ok=True)
            kwargs["tmpdir"] = tmpdir

    res = run_bass_kernel_spmd(
        nc,
        in_maps,
        list(range(NCORES)),
        trace=trace,
        **kwargs,
    )
    _last_results = res

    parts = [res.results[c]["out"].reshape(TOK, OSH) for c in range(NCORES)]
    return np.concatenate(parts, axis=1).reshape(B, S, OUT)
